# revision 31
# baseline (speedup 1.0000x reference)
"""Trainium2 Bass kernel for nn_GTLayer (sparse_attention problem).

Key structural fact about the reference: H == 1 and the softmax is taken
over the HEAD axis, so softmax(attn, axis=0) on a (1, N, N) tensor is
identically 1.0.  Therefore attn @ v reduces to broadcasting the column
sums of v to every row: the A mask, q and k projections are all dead
code.  The attention output row is a single constant vector

    base = (sum_i h_i) @ vw + N * vb, then @ ow + ob

which we compute exactly on the host.  Folding both BatchNorms (eval
mode -> per-feature affine) and the residuals, the whole layer is

    y = h2 + relu(h2 @ W1 + b1) @ W2 + Cfull      (per-feature constants)

with h2 = h * sP.  The device computes only the non-constant FFN part

    F = tv @ W2,   tv = relu(z + b1) - relu(b1)

in fp8 e4m3 with DoubleRow matmuls (2 MACs/cell/cycle, ~216ns per
256-contraction x 512-free matmul).  fp8 error is diluted ~1000x
because the output is dominated by the exactly-computed h2 + Cfull part
added on the host (measured rel err ~1e-4 vs the 2e-2 gate).

tv trick: the hidden units are PERMUTED on the host so b1<=0 units come
first.  Then per 128-chunk:
    b1 <= 0 chunk:  tv = relu(z + b1)        -> one ScalarE activation
    b1 >  0 chunk:  tv = max(z, -b1)         -> one VectorE tensor_scalar
    mixed boundary: tv = max(z+min(b1,0), -relu(b1)) -> two-op tensor_scalar
splitting the psum->fp8 conversion work across both engines (the
permutation commutes through the FFN since W2 rows are permuted too).

Scaling: weights carry power-of-2 scales (W1*32, W2*512) chosen so
psum1 = 32*z arrives already at fp8-friendly scale for tv (|32*tv| <=
~120 < 240 = TRN e4m3 max); psum2 = 16384*F is scaled back to bf16 by
the output copy (ScalarE Copy / VectorE mult, alternating).

Rows (N=8192) are sharded over the 8 cores; weights are replicated.
"""

import numpy as np
from contextlib import ExitStack

import ml_dtypes
import concourse.bass as bass
import concourse.mybir as mybir
import concourse.tile as tile
from concourse import bacc
from concourse.bass_utils import run_bass_kernel_spmd

N = 8192
D = 512
H1 = 1024
NCORES = 8
RPC = N // NCORES  # rows per core
EPS = 1e-5
N_WARMUP = 11
S1 = 32.0    # W1 scale (so psum1 = 32*z)
S3 = 512.0   # W2 scale
S23 = S1 * S3

BF16 = mybir.dt.bfloat16
F32 = mybir.dt.float32
FP8 = mybir.dt.float8e4
NPBF16 = np.dtype(ml_dtypes.bfloat16)
NPFP8 = np.dtype(ml_dtypes.float8_e4m3)

KC = D // 128    # 4 contraction chunks in mm1
NC = H1 // 128   # 8 n chunks (mm1 out / mm2 contraction)
DC = D // 128    # 4 d chunks (mm2 out)
RG = RPC // 512  # 2 row groups (matmul free dim 512)


def build_bass(pattern):
    """pattern: per-128-chunk tv-op kind: 'a' = ACT relu (b1<=0 chunk),
    'v' = DVE single max (b1>0 chunk), 'm' = DVE two-op (mixed chunk)."""
    nc = bacc.Bacc(
        "TRN2", target_bir_lowering=False, debug=False, num_devices=NCORES
    )
    h8 = nc.dram_tensor("h8", [D, RPC], FP8, kind="ExternalInput")
    w1 = nc.dram_tensor("w1", [D, H1], FP8, kind="ExternalInput")
    w2 = nc.dram_tensor("w2", [H1, D], FP8, kind="ExternalInput")
    # min(b1,0) (cols 0..7) and -relu(b1) (cols 8..15) packed
    bc = nc.dram_tensor("bc", [128, 2 * NC], F32, kind="ExternalInput")
    y = nc.dram_tensor("y", [D, RPC], BF16, kind="ExternalOutput")

    DRM = mybir.MatmulPerfMode.DoubleRow

    with ExitStack() as ctx:
        tc = ctx.enter_context(tile.TileContext(nc))
        consts = ctx.enter_context(tc.tile_pool(name="consts", bufs=1))
        acts = ctx.enter_context(tc.tile_pool(name="acts", bufs=1))
        zpsum = ctx.enter_context(tc.tile_pool(name="zpsum", bufs=4, space="PSUM"))
        fpsum = ctx.enter_context(tc.tile_pool(name="fpsum", bufs=4, space="PSUM"))
        ypool = ctx.enter_context(tc.tile_pool(name="ypool", bufs=4))

        # --- PE warm-up on a memset tile: no DMA dependency, so the PE's
        # HAM activity window fills right after the preamble and real
        # matmuls run at 2.4 GHz instead of 1.2.  Shares the "fp" psum
        # ring so it costs no extra PSUM bank.
        wa = consts.tile([128, 512], BF16)
        nc.vector.memset(wa[:], 0.0)
        wp = fpsum.tile([128, 512], F32, tag="fp", name="wp")
        for _ in range(N_WARMUP):
            nc.tensor.matmul(wp[:], wa[:, :128], wa[:], start=True, stop=True)

        # --- streaming inputs, critical-path order, few triggers ----------
        w1sb = consts.tile([128, KC, H1], FP8)
        h2sb = acts.tile([128, KC, RPC], FP8)
        w2sb = consts.tile([128, NC, D], FP8)
        bcsb = consts.tile([128, 2 * NC], F32)
        tvsb = acts.tile([128, NC, RPC], FP8)

        W1r = w1.rearrange("(kc p) n -> p kc n", p=128)
        H8r = h8.rearrange("(kc p) r -> p kc r", p=128)
        W2r = w2.rearrange("(c p) d -> p c d", p=128)
        # weights stream on the sync queue, activations on the scalar
        # queue: one hardware DMA queue only sustains ~165 GB/s, so the
        # two critical streams (w1 kc-half + h8 kc-half, each needed by
        # the software-pipelined mm1) transfer in PARALLEL
        nc.sync.dma_start(w1sb[:, 0:2, :], W1r[:, 0:2, :])
        nc.scalar.dma_start(h2sb[:, 0:2, :], H8r[:, 0:2, :])
        nc.sync.dma_start(w1sb[:, 2:4, :], W1r[:, 2:4, :])
        nc.scalar.dma_start(h2sb[:, 2:4, :], H8r[:, 2:4, :])
        nc.sync.dma_start(w2sb[:, 0:4, :], W2r[:, 0:4, :])
        nc.scalar.dma_start(w2sb[:, 4:8, :], W2r[:, 4:8, :])
        # tiny but needed by the FIRST tv op: gpsimd software-DGE queue so
        # it lands immediately instead of behind the h8 stream
        nc.gpsimd.dma_start(bcsb[:], bc[:, :])

        def tv_op(nci, rg, zp):
            dst = tvsb[:, nci, rg * 512 : (rg + 1) * 512]
            kind = pattern[nci]
            if kind == "a":
                nc.scalar.activation(
                    dst,
                    zp[:],
                    mybir.ActivationFunctionType.Relu,
                    bias=bcsb[:, nci : nci + 1],
                    scale=1.0,
                )
            elif kind == "v":
                nc.vector.tensor_scalar(
                    dst,
                    zp[:],
                    bcsb[:, NC + nci : NC + nci + 1],
                    None,
                    mybir.AluOpType.max,
                )
            else:
                nc.vector.tensor_scalar(
                    dst,
                    zp[:],
                    bcsb[:, nci : nci + 1],
                    bcsb[:, NC + nci : NC + nci + 1],
                    mybir.AluOpType.add,
                    mybir.AluOpType.max,
                )

        # --- mm1: per nci one psum group per row-group, 2 DoubleRow
        # matmuls each (256-contraction), weight chunk reused across both
        # row groups (1 LDW per 2 MMs keeps the 216ns/MM stream rate).
        # Software-pipelined: group nci's second contraction half (i=1,
        # needs w1/h8 kc 2:4) is emitted after group nci+1's first half,
        # so the PE can stream on the first DMA chunk ~1us earlier.
        zps = {}

        def mm1_half(nci, i):
            ns = slice(nci * 128, (nci + 1) * 128)
            ks = slice(2 * i, 2 * i + 2)
            for rg in range(RG):
                nc.tensor.matmul(
                    zps[nci][rg][:],
                    w1sb[:, ks, ns],
                    h2sb[:, ks, rg * 512 : (rg + 1) * 512],
                    start=(i == 0),
                    stop=(i == KC // 2 - 1),
                    perf_mode=DRM,
                )

        for nci in range(NC):
            zps[nci] = [
                zpsum.tile([128, 512], F32, tag="zp", name=f"zp{nci}_{g}")
                for g in range(RG)
            ]
            mm1_half(nci, 0)
            if nci >= 1:
                mm1_half(nci - 1, 1)
                for rg in range(RG):
                    tv_op(nci - 1, rg, zps[nci - 1][rg])
        mm1_half(NC - 1, 1)
        for rg in range(RG):
            tv_op(NC - 1, rg, zps[NC - 1][rg])

        # --- mm2: W2 stationary, output F^T tiles [d-chunk, rows];
        # rg-interleaved per dc so each weight chunk's LDW feeds 2 MMs.
        Yr = y.rearrange("(dc p) r -> dc p r", p=128)
        for dc in range(DC):
            ds = slice(dc * 128, (dc + 1) * 128)
            fps = [
                fpsum.tile([128, 512], F32, tag="fp", name=f"fp{dc}_{g}")
                for g in range(RG)
            ]
            for i in range(NC // 2):
                ks = slice(2 * i, 2 * i + 2)
                for rg in range(RG):
                    nc.tensor.matmul(
                        fps[rg][:],
                        w2sb[:, ks, ds],
                        tvsb[:, ks, rg * 512 : (rg + 1) * 512],
                        start=(i == 0),
                        stop=(i == NC // 2 - 1),
                        perf_mode=DRM,
                    )
            for rg in range(RG):
                rs = slice(rg * 512, (rg + 1) * 512)
                ysb = ypool.tile([128, 512], BF16, tag="ysb", name=f"y{dc}_{rg}")
                if dc == DC - 1:
                    # final dc: rg0 on DVE + sync trigger, rg1 on ACT +
                    # scalar trigger -> both copies AND both triggers run
                    # in parallel right as the psum groups close
                    if rg == 0:
                        nc.vector.tensor_scalar(
                            ysb[:], fps[rg][:], 1.0 / S23, None, mybir.AluOpType.mult
                        )
                        nc.sync.dma_start(Yr[dc, :, rs], ysb[:])
                    else:
                        nc.scalar.activation(
                            ysb[:],
                            fps[rg][:],
                            mybir.ActivationFunctionType.Copy,
                            bias=0.0,
                            scale=1.0 / S23,
                        )
                        nc.scalar.dma_start(Yr[dc, :, rs], ysb[:])
                elif rg == 0:
                    # ACT copy + scalar-queue trigger: keeps all of this
                    # tile's work off the sync queue (which carries the
                    # rg1 tiles), so output triggers never pile up
                    nc.scalar.activation(
                        ysb[:],
                        fps[rg][:],
                        mybir.ActivationFunctionType.Copy,
                        bias=0.0,
                        scale=1.0 / S23,
                    )
                    nc.scalar.dma_start(Yr[dc, :, rs], ysb[:])
                else:
                    nc.vector.tensor_scalar(
                        ysb[:], fps[rg][:], 1.0 / S23, None, mybir.AluOpType.mult
                    )
                    nc.sync.dma_start(Yr[dc, :, rs], ysb[:])
    nc.compile()
    return nc


_CACHE = {}


def _get_bass(cb):
    if cb not in _CACHE:
        _CACHE[cb] = build_bass(cb)
    return _CACHE[cb]


def _host_fold(inputs):
    """Fold attention shortcut + BNs into W1, b1, W2, h2, Cfull (float64)."""
    f = lambda k: inputs[k].astype(np.float64)
    h = f("h")
    a1 = f("bn1_g") / np.sqrt(f("bn1_v") + EPS)
    c1 = f("bn1_b") - f("bn1_m") * a1
    a2 = f("bn2_g") / np.sqrt(f("bn2_v") + EPS)
    c2 = f("bn2_b") - f("bn2_m") * a2

    hs = h.sum(axis=0)
    s = hs @ f("vw") + N * f("vb")          # column sums of v
    base = s @ f("ow") + f("ob")            # constant attention-out row
    d1 = base * a1 + c1                     # constant row of bn1(x)
    sP = a1 * a2

    W1 = (1.0 / a2)[:, None] * f("f1w")
    b1 = d1 @ f("f1w") + f("f1b")
    W2 = f("f2w") * a2[None, :]
    C = (d1 + f("f2b")) * a2 + c2

    h2 = h * sP[None, :]

    # permute hidden units into sign-pure 128-chunks, ALTERNATING
    # neg/pos chunks so consecutive tv tiles land on different engines
    # (ScalarE for b1<=0 relu chunks, VectorE for b1>0 max chunks); the
    # one mixed-sign leftover chunk goes last (two-op VectorE)
    neg = np.flatnonzero(b1 <= 0)
    pos = np.flatnonzero(b1 > 0)
    na, nv = len(neg) // 128, len(pos) // 128
    chunks, pattern = [], []
    ia = iv = 0
    for k in range(na + nv):
        # take from whichever side is proportionally more behind
        take_neg = iv >= nv or (ia < na and ia * nv <= iv * na)
        if take_neg:
            chunks.append(neg[ia * 128 : (ia + 1) * 128]); pattern.append("a")
            ia += 1
        else:
            chunks.append(pos[iv * 128 : (iv + 1) * 128]); pattern.append("v")
            iv += 1
    rest = np.concatenate([neg[na * 128 :], pos[nv * 128 :]])
    if len(rest):
        chunks.append(rest); pattern.append("m")
    order = np.concatenate(chunks)
    pattern = tuple(pattern)
    assert len(order) == H1 and len(pattern) == NC
    W1 = W1[:, order]
    W2 = W2[order, :]
    b1 = b1[order]

    b1p = (S1 * b1).astype(np.float32)
    b1n = np.minimum(b1p, 0.0)
    mtc = -np.maximum(b1p, 0.0)
    # device computes tv with the exact f32 constants above; fold the
    # same f32 tc into the constant so host+device agree bit-for-bit
    Cfull = C + (np.maximum(b1p, 0.0).astype(np.float64) / S1) @ W2

    pack = lambda v: np.ascontiguousarray(v.reshape(NC, 128).T)
    return {
        "pattern": pattern,
        "w1": np.ascontiguousarray((W1 * S1).astype(NPFP8)),
        "w2": np.ascontiguousarray((W2 * S3).astype(NPFP8)),
        "bc": np.ascontiguousarray(
            np.concatenate([pack(b1n), pack(mtc)], axis=1).astype(np.float32)
        ),
        "h2": h2,
        "hC": (h2 + Cfull[None, :]).astype(np.float32),
    }


def make_in_maps(inputs):
    hf = _host_fold(inputs)
    in_maps = []
    for c in range(NCORES):
        r0 = c * RPC
        in_maps.append(
            {
                "h8": np.ascontiguousarray(hf["h2"][r0 : r0 + RPC].T).astype(NPFP8),
                "w1": hf["w1"],
                "w2": hf["w2"],
                "bc": hf["bc"],
            }
        )
    return in_maps, hf


def kernel(**inputs):
    in_maps, hf = make_in_maps(inputs)
    nc = _get_bass(hf["pattern"])
    res = run_bass_kernel_spmd(nc, in_maps, core_ids=list(range(NCORES)))
    out = np.empty((N, D), np.float32)
    for c in range(NCORES):
        r0 = c * RPC
        out[r0 : r0 + RPC] = res.results[c]["y"].astype(np.float32).T
    out += hf["hC"]
    return out


# revision 34
# speedup vs baseline: 1.0028x; 1.0028x over previous
"""Trainium2 Bass kernel for nn_GTLayer (sparse_attention problem).

Key structural fact about the reference: H == 1 and the softmax is taken
over the HEAD axis, so softmax(attn, axis=0) on a (1, N, N) tensor is
identically 1.0.  Therefore attn @ v reduces to broadcasting the column
sums of v to every row: the A mask, q and k projections are all dead
code.  The attention output row is a single constant vector

    base = (sum_i h_i) @ vw + N * vb, then @ ow + ob

which we compute exactly on the host.  Folding both BatchNorms (eval
mode -> per-feature affine) and the residuals, the whole layer is

    y = h2 + relu(h2 @ W1 + b1) @ W2 + Cfull      (per-feature constants)

with h2 = h * sP.  The device computes only the non-constant FFN part

    F = tv @ W2,   tv = relu(z + b1) - relu(b1)

in fp8 e4m3 with DoubleRow matmuls (2 MACs/cell/cycle, ~216ns per
256-contraction x 512-free matmul).  fp8 error is diluted ~1000x
because the output is dominated by the exactly-computed h2 + Cfull part
added on the host (measured rel err ~1e-4 vs the 2e-2 gate).

tv trick: the hidden units are PERMUTED on the host so b1<=0 units come
first.  Then per 128-chunk:
    b1 <= 0 chunk:  tv = relu(z + b1)        -> one ScalarE activation
    b1 >  0 chunk:  tv = max(z, -b1)         -> one VectorE tensor_scalar
    mixed boundary: tv = max(z+min(b1,0), -relu(b1)) -> two-op tensor_scalar
splitting the psum->fp8 conversion work across both engines (the
permutation commutes through the FFN since W2 rows are permuted too).

Scaling: weights carry power-of-2 scales (W1*32, W2*512) chosen so
psum1 = 32*z arrives already at fp8-friendly scale for tv (|32*tv| <=
~120 < 240 = TRN e4m3 max); psum2 = 16384*F is scaled back to bf16 by
the output copy (ScalarE Copy / VectorE mult, alternating).

Rows (N=8192) are sharded over the 8 cores; weights are replicated.
"""

import numpy as np
from contextlib import ExitStack

import ml_dtypes
import concourse.bass as bass
import concourse.mybir as mybir
import concourse.tile as tile
from concourse import bacc
from concourse.bass_utils import run_bass_kernel_spmd

N = 8192
D = 512
H1 = 1024
NCORES = 8
RPC = N // NCORES  # rows per core
EPS = 1e-5
N_WARMUP = 9  # 512-free warmup matmuls, then finer 64-free ones
S1 = 32.0    # W1 scale (so psum1 = 32*z)
S3 = 512.0   # W2 scale
S23 = S1 * S3

BF16 = mybir.dt.bfloat16
F32 = mybir.dt.float32
FP8 = mybir.dt.float8e4
NPBF16 = np.dtype(ml_dtypes.bfloat16)
NPFP8 = np.dtype(ml_dtypes.float8_e4m3)

KC = D // 128    # 4 contraction chunks in mm1
NC = H1 // 128   # 8 n chunks (mm1 out / mm2 contraction)
DC = D // 128    # 4 d chunks (mm2 out)
RG = RPC // 512  # 2 row groups (matmul free dim 512)


def build_bass(pattern):
    """pattern: per-128-chunk tv-op kind: 'a' = ACT relu (b1<=0 chunk),
    'v' = DVE single max (b1>0 chunk), 'm' = DVE two-op (mixed chunk)."""
    nc = bacc.Bacc(
        "TRN2", target_bir_lowering=False, debug=False, num_devices=NCORES
    )
    h8 = nc.dram_tensor("h8", [D, RPC], FP8, kind="ExternalInput")
    w1 = nc.dram_tensor("w1", [D, H1], FP8, kind="ExternalInput")
    w2 = nc.dram_tensor("w2", [H1, D], FP8, kind="ExternalInput")
    # min(b1,0) (cols 0..7) and -relu(b1) (cols 8..15) packed
    bc = nc.dram_tensor("bc", [128, 2 * NC], F32, kind="ExternalInput")
    y = nc.dram_tensor("y", [D, RPC], BF16, kind="ExternalOutput")

    DRM = mybir.MatmulPerfMode.DoubleRow

    with ExitStack() as ctx:
        tc = ctx.enter_context(tile.TileContext(nc))
        consts = ctx.enter_context(tc.tile_pool(name="consts", bufs=1))
        acts = ctx.enter_context(tc.tile_pool(name="acts", bufs=1))
        zpsum = ctx.enter_context(tc.tile_pool(name="zpsum", bufs=5, space="PSUM"))
        fpsum = ctx.enter_context(tc.tile_pool(name="fpsum", bufs=3, space="PSUM"))
        ypool = ctx.enter_context(tc.tile_pool(name="ypool", bufs=4))

        # --- PE warm-up on a memset tile: no DMA dependency, so the PE's
        # HAM activity window fills right after the preamble and real
        # matmuls run at 2.4 GHz instead of 1.2.  Shares the "fp" psum
        # ring so it costs no extra PSUM bank.
        wa = consts.tile([128, 512], BF16)
        nc.vector.memset(wa[:], 0.0)
        wp = fpsum.tile([128, 512], F32, tag="fp", name="wp")
        for _ in range(N_WARMUP):
            nc.tensor.matmul(wp[:], wa[:, :128], wa[:], start=True, stop=True)
        # finer-granularity warmup tail: the first real matmul starts at
        # most ~107ns after its data lands instead of ~427ns
        for _ in range(8):
            nc.tensor.matmul(wp[:, :64], wa[:, :128], wa[:, :64], start=True, stop=True)

        # --- streaming inputs, critical-path order, few triggers ----------
        w1sb = consts.tile([128, KC, H1], FP8)
        h2sb = acts.tile([128, KC, RPC], FP8)
        w2sb = consts.tile([128, NC, D], FP8)
        bcsb = consts.tile([128, 2 * NC], F32)
        tvsb = acts.tile([128, NC, RPC], FP8)

        W1r = w1.rearrange("(kc p) n -> p kc n", p=128)
        H8r = h8.rearrange("(kc p) r -> p kc r", p=128)
        W2r = w2.rearrange("(c p) d -> p c d", p=128)
        # weights stream on the sync queue, activations on the scalar
        # queue: one hardware DMA queue only sustains ~165 GB/s, so the
        # two critical streams (w1 kc-half + h8 kc-half, each needed by
        # the software-pipelined mm1) transfer in PARALLEL
        nc.sync.dma_start(w1sb[:, 0:2, :], W1r[:, 0:2, :])
        nc.scalar.dma_start(h2sb[:, 0:2, :], H8r[:, 0:2, :])
        nc.sync.dma_start(w1sb[:, 2:4, :], W1r[:, 2:4, :])
        nc.scalar.dma_start(h2sb[:, 2:4, :], H8r[:, 2:4, :])
        nc.sync.dma_start(w2sb[:, 0:4, :], W2r[:, 0:4, :])
        nc.scalar.dma_start(w2sb[:, 4:8, :], W2r[:, 4:8, :])
        # tiny but needed by the FIRST tv op: gpsimd software-DGE queue so
        # it lands immediately instead of behind the h8 stream
        nc.gpsimd.dma_start(bcsb[:], bc[:, :])

        def tv_op(nci, rg, zp):
            dst = tvsb[:, nci, rg * 512 : (rg + 1) * 512]
            kind = pattern[nci]
            if kind == "a":
                nc.scalar.activation(
                    dst,
                    zp[:],
                    mybir.ActivationFunctionType.Relu,
                    bias=bcsb[:, nci : nci + 1],
                    scale=1.0,
                )
            elif kind == "v":
                nc.vector.tensor_scalar(
                    dst,
                    zp[:],
                    bcsb[:, NC + nci : NC + nci + 1],
                    None,
                    mybir.AluOpType.max,
                )
            else:
                nc.vector.tensor_scalar(
                    dst,
                    zp[:],
                    bcsb[:, nci : nci + 1],
                    bcsb[:, NC + nci : NC + nci + 1],
                    mybir.AluOpType.add,
                    mybir.AluOpType.max,
                )

        # --- mm1: per nci one psum group per row-group, 2 DoubleRow
        # matmuls each (256-contraction), weight chunk reused across both
        # row groups (1 LDW per 2 MMs keeps the 216ns/MM stream rate).
        # Software-pipelined: group nci's second contraction half (i=1,
        # needs w1/h8 kc 2:4) is emitted after group nci+1's first half,
        # so the PE can stream on the first DMA chunk ~1us earlier.
        zps = {}

        def mm1_half(nci, i):
            ns = slice(nci * 128, (nci + 1) * 128)
            ks = slice(2 * i, 2 * i + 2)
            for rg in range(RG):
                nc.tensor.matmul(
                    zps[nci][rg][:],
                    w1sb[:, ks, ns],
                    h2sb[:, ks, rg * 512 : (rg + 1) * 512],
                    start=(i == 0),
                    stop=(i == KC // 2 - 1),
                    perf_mode=DRM,
                )

        for nci in range(NC):
            zps[nci] = [
                zpsum.tile([128, 512], F32, tag="zp", name=f"zp{nci}_{g}")
                for g in range(RG)
            ]
            mm1_half(nci, 0)
            if nci >= 1:
                mm1_half(nci - 1, 1)
                for rg in range(RG):
                    tv_op(nci - 1, rg, zps[nci - 1][rg])
        mm1_half(NC - 1, 1)
        for rg in range(RG):
            tv_op(NC - 1, rg, zps[NC - 1][rg])

        # --- mm2: W2 stationary, output F^T tiles [d-chunk, rows];
        # rg-interleaved per dc so each weight chunk's LDW feeds 2 MMs.
        Yr = y.rearrange("(dc p) r -> dc p r", p=128)
        for dc in range(DC):
            ds = slice(dc * 128, (dc + 1) * 128)
            fps = [
                fpsum.tile([128, 512], F32, tag="fp", name=f"fp{dc}_{g}")
                for g in range(RG)
            ]
            for i in range(NC // 2):
                ks = slice(2 * i, 2 * i + 2)
                for rg in range(RG):
                    nc.tensor.matmul(
                        fps[rg][:],
                        w2sb[:, ks, ds],
                        tvsb[:, ks, rg * 512 : (rg + 1) * 512],
                        start=(i == 0),
                        stop=(i == NC // 2 - 1),
                        perf_mode=DRM,
                    )
            for rg in range(RG):
                rs = slice(rg * 512, (rg + 1) * 512)
                ysb = ypool.tile([128, 512], BF16, tag="ysb", name=f"y{dc}_{rg}")
                if dc == DC - 1:
                    # final dc: rg0 on DVE + sync trigger, rg1 on ACT +
                    # scalar trigger -> both copies AND both triggers run
                    # in parallel right as the psum groups close
                    if rg == 0:
                        nc.vector.tensor_scalar(
                            ysb[:], fps[rg][:], 1.0 / S23, None, mybir.AluOpType.mult
                        )
                        nc.sync.dma_start(Yr[dc, :, rs], ysb[:])
                    else:
                        nc.scalar.activation(
                            ysb[:],
                            fps[rg][:],
                            mybir.ActivationFunctionType.Copy,
                            bias=0.0,
                            scale=1.0 / S23,
                        )
                        nc.scalar.dma_start(Yr[dc, :, rs], ysb[:])
                elif rg == 0:
                    # ACT copy + scalar-queue trigger: keeps all of this
                    # tile's work off the sync queue (which carries the
                    # rg1 tiles), so output triggers never pile up
                    nc.scalar.activation(
                        ysb[:],
                        fps[rg][:],
                        mybir.ActivationFunctionType.Copy,
                        bias=0.0,
                        scale=1.0 / S23,
                    )
                    nc.scalar.dma_start(Yr[dc, :, rs], ysb[:])
                else:
                    nc.vector.tensor_scalar(
                        ysb[:], fps[rg][:], 1.0 / S23, None, mybir.AluOpType.mult
                    )
                    nc.sync.dma_start(Yr[dc, :, rs], ysb[:])
    nc.compile()
    return nc


_CACHE = {}


def _get_bass(cb):
    if cb not in _CACHE:
        _CACHE[cb] = build_bass(cb)
    return _CACHE[cb]


def _host_fold(inputs):
    """Fold attention shortcut + BNs into W1, b1, W2, h2, Cfull (float64)."""
    f = lambda k: inputs[k].astype(np.float64)
    h = f("h")
    a1 = f("bn1_g") / np.sqrt(f("bn1_v") + EPS)
    c1 = f("bn1_b") - f("bn1_m") * a1
    a2 = f("bn2_g") / np.sqrt(f("bn2_v") + EPS)
    c2 = f("bn2_b") - f("bn2_m") * a2

    hs = h.sum(axis=0)
    s = hs @ f("vw") + N * f("vb")          # column sums of v
    base = s @ f("ow") + f("ob")            # constant attention-out row
    d1 = base * a1 + c1                     # constant row of bn1(x)
    sP = a1 * a2

    W1 = (1.0 / a2)[:, None] * f("f1w")
    b1 = d1 @ f("f1w") + f("f1b")
    W2 = f("f2w") * a2[None, :]
    C = (d1 + f("f2b")) * a2 + c2

    h2 = h * sP[None, :]

    # permute hidden units into sign-pure 128-chunks, ALTERNATING
    # neg/pos chunks so consecutive tv tiles land on different engines
    # (ScalarE for b1<=0 relu chunks, VectorE for b1>0 max chunks); the
    # one mixed-sign leftover chunk goes last (two-op VectorE)
    neg = np.flatnonzero(b1 <= 0)
    pos = np.flatnonzero(b1 > 0)
    na, nv = len(neg) // 128, len(pos) // 128
    chunks, pattern = [], []
    ia = iv = 0
    for k in range(na + nv):
        # take from whichever side is proportionally more behind
        take_neg = iv >= nv or (ia < na and ia * nv <= iv * na)
        if take_neg:
            chunks.append(neg[ia * 128 : (ia + 1) * 128]); pattern.append("a")
            ia += 1
        else:
            chunks.append(pos[iv * 128 : (iv + 1) * 128]); pattern.append("v")
            iv += 1
    rest = np.concatenate([neg[na * 128 :], pos[nv * 128 :]])
    if len(rest):
        chunks.append(rest); pattern.append("m")
    order = np.concatenate(chunks)
    pattern = tuple(pattern)
    assert len(order) == H1 and len(pattern) == NC
    W1 = W1[:, order]
    W2 = W2[order, :]
    b1 = b1[order]

    b1p = (S1 * b1).astype(np.float32)
    b1n = np.minimum(b1p, 0.0)
    mtc = -np.maximum(b1p, 0.0)
    # device computes tv with the exact f32 constants above; fold the
    # same f32 tc into the constant so host+device agree bit-for-bit
    Cfull = C + (np.maximum(b1p, 0.0).astype(np.float64) / S1) @ W2

    pack = lambda v: np.ascontiguousarray(v.reshape(NC, 128).T)
    return {
        "pattern": pattern,
        "w1": np.ascontiguousarray((W1 * S1).astype(NPFP8)),
        "w2": np.ascontiguousarray((W2 * S3).astype(NPFP8)),
        "bc": np.ascontiguousarray(
            np.concatenate([pack(b1n), pack(mtc)], axis=1).astype(np.float32)
        ),
        "h2": h2,
        "hC": (h2 + Cfull[None, :]).astype(np.float32),
    }


def make_in_maps(inputs):
    hf = _host_fold(inputs)
    in_maps = []
    for c in range(NCORES):
        r0 = c * RPC
        in_maps.append(
            {
                "h8": np.ascontiguousarray(hf["h2"][r0 : r0 + RPC].T).astype(NPFP8),
                "w1": hf["w1"],
                "w2": hf["w2"],
                "bc": hf["bc"],
            }
        )
    return in_maps, hf


def kernel(**inputs):
    in_maps, hf = make_in_maps(inputs)
    nc = _get_bass(hf["pattern"])
    res = run_bass_kernel_spmd(nc, in_maps, core_ids=list(range(NCORES)))
    out = np.empty((N, D), np.float32)
    for c in range(NCORES):
        r0 = c * RPC
        out[r0 : r0 + RPC] = res.results[c]["y"].astype(np.float32).T
    out += hf["hC"]
    return out
